# revision 1
# baseline (speedup 1.0000x reference)
"""Mamba block (RMSNorm -> in_proj -> causal conv -> selective scan -> gate
-> out_proj -> residual) on 8 Trainium2 NeuronCores.

Sharding: d_inner (4096) channel-parallel across 8 cores (512 ch/core).
Two SPMD launches:
  phase 1: rmsnorm + in_proj (both halves) + conv + silu + partial x_proj
  (host) : sum partial x_dbl across cores (the "all-reduce"), split
           delta_low/B/C, replicate B/C across partitions
  phase 2: dt_proj + softplus + selective scan (tensor_tensor_scan with
           (d,n)-pairs on partitions, time on the free axis) + gate +
           partial out_proj
  (host) : sum partial out_proj across cores, add residual.

Scan layout: partitions hold 128 (d,n) pairs (d-major), free dim is t.
delta/du are replicated across the 16 n's per d with 0/1 "replication"
matmuls on the PE; exp(A*delta) runs on the scalar engine with per-
partition scale A; the n-reduction of h*C runs as 0/1 "selection"
matmuls accumulated in PSUM.
"""

import sys

if '/opt/trn_rl_repo' not in sys.path:
    sys.path.insert(0, '/opt/trn_rl_repo')

import numpy as np

import concourse.bass as bass
import concourse.tile as tile
from concourse import mybir
from concourse.bass_utils import run_bass_kernel_spmd
from concourse.vector_clock import ScopedClock

# ----------------------------------------------------------------------------
# Workaround: this walrus build rejects a Drain instruction carrying more than
# one semaphore wait. Split the TileContext tail-drain waits across multiple
# consecutive SP drains (semantically identical: all waits complete before the
# following all-engine barrier).
_MAX_DRAIN_WAITS = 1


def _patched_drain_and_barrier(self, tick_clock, wait_clock):
    nc = self.nc
    drain_inst = nc.sync.drain()
    wait_clock.add_sem_waits(
        drain_inst.ins, ScopedClock({None: tick_clock.global_clock})
    )
    si = drain_inst.ins.sync_info
    if si is not None and len(si.on_wait) > _MAX_DRAIN_WAITS:
        waits = list(si.on_wait)
        del si.on_wait[_MAX_DRAIN_WAITS:]
        rest = waits[_MAX_DRAIN_WAITS:]
        while rest:
            d2 = nc.sync.drain()
            chunk, rest = rest[:_MAX_DRAIN_WAITS], rest[_MAX_DRAIN_WAITS:]
            si2 = d2.ins.sync_info
            if si2 is None:
                d2.ins.sync_info = type(si)(on_wait=list(chunk), on_update=[])
            else:
                si2.on_wait.extend(chunk)

    nc.all_engine_barrier()
    assert self.sems is not None
    popped = nc._tile_sem_poison_stack.pop()
    assert popped is self._sem_poison
    nc.clear_and_free_semaphores(list(self.sems.allocated().values()))
    nc.all_engine_barrier()


tile.TileContext._drain_and_barrier = _patched_drain_and_barrier


def _split_sync_waits(nc):
    """This walrus build rejects >1 sync wait per instruction; hoist extra
    waits onto same-engine NOPs inserted immediately before."""
    for fn in nc.m.functions:
        for bb in fn.blocks:
            new = []
            for inst in bb.instructions:
                si = inst.sync_info
                if si is not None and len(si.on_wait) > 1:
                    waits = list(si.on_wait)
                    del si.on_wait[:-1]
                    for w in waits[:-1]:
                        nop = mybir.InstNoOp(
                            name=nc.get_next_instruction_name(),
                            engine=inst.engine,
                            sync_info=mybir.SyncInfo(on_wait=[w],
                                                     on_update=[]),
                            bass_nofuse=True,
                        )
                        nc.register_instruction(nop)
                        new.append(nop)
                new.append(inst)
            bb.instructions[:] = new
# ----------------------------------------------------------------------------

NCORES = 8
L = 1024          # sequence length (b=1)
DMODEL = 2048     # d_model
DIN = 4096        # d_inner
NST = 16          # ssm state size n
DCONV = 4
DTR = 128         # dt_rank
DL = DIN // NCORES  # 512 channels per core
EPS = 1e-5

F32 = mybir.dt.float32
F32R = mybir.dt.float32r
BF16 = mybir.dt.bfloat16
AF = mybir.ActivationFunctionType
OP = mybir.AluOpType


def _r(ap):
    return ap.bitcast(F32R)


def _new_nc():
    return bass.Bass("TRN2", target_bir_lowering=False, debug=False,
                     num_devices=NCORES)


# ============================================================================
# Phase 1
# ============================================================================

def _build_phase1():
    nc = _new_nc()
    xt = nc.dram_tensor("xt", [DMODEL, L], BF16, kind="ExternalInput").ap()
    w1t = nc.dram_tensor("w1t", [DMODEL, DL], BF16, kind="ExternalInput").ap()
    w2t = nc.dram_tensor("w2t", [DMODEL, DL], BF16, kind="ExternalInput").ap()
    xpt = nc.dram_tensor("xpt", [DL, 160], BF16, kind="ExternalInput").ap()
    cwt = nc.dram_tensor("cwt", [128, 16], F32, kind="ExternalInput").ap()
    cbt = nc.dram_tensor("cbt", [128, 4], F32, kind="ExternalInput").ap()
    ones_r = nc.dram_tensor("ones_r", [1, 128], F32, kind="ExternalInput").ap()
    ones_c = nc.dram_tensor("ones_c", [128, 1], F32, kind="ExternalInput").ap()
    xc_out = nc.dram_tensor("xc_out", [DL, L], BF16, kind="ExternalOutput").ap()
    g_out = nc.dram_tensor("g_out", [DL, L], BF16, kind="ExternalOutput").ap()
    xdp_out = nc.dram_tensor("xdp_out", [160, L], F32, kind="ExternalOutput").ap()

    KT = DMODEL // 128  # 16 K-tiles

    with tile.TileContext(nc) as tc:
        with (
            tc.tile_pool(name="px", bufs=1) as px,
            tc.tile_pool(name="pw", bufs=1) as pw,
            tc.tile_pool(name="pc", bufs=1) as pc,
            tc.tile_pool(name="psq", bufs=2) as psq,
            tc.tile_pool(name="pxz", bufs=2) as pxz,
            tc.tile_pool(name="pcv", bufs=2) as pcv,
            tc.tile_pool(name="pxc", bufs=4) as pxc,
            tc.tile_pool(name="pres", bufs=2) as pres,
            tc.tile_pool(name="pxd", bufs=2) as pxd,
            tc.tile_pool(name="pp", bufs=2, space="PSUM") as pp,
            tc.tile_pool(name="pps", bufs=1, space="PSUM") as pps,
            tc.tile_pool(name="ppb", bufs=2, space="PSUM") as ppb,
        ):
            w1 = pw.tile([128, KT, DL], BF16, tag="w")
            nc.sync.dma_start(
                w1[:], w1t.rearrange("(k p) m -> p k m", p=128))
            xsb = px.tile([128, KT, L], BF16)
            xt_r = xt.rearrange("(k p) t -> p k t", p=128)
            for ch in range(4):
                nc.sync.dma_start(xsb[:, 4 * ch:4 * (ch + 1), :],
                                  xt_r[:, 4 * ch:4 * (ch + 1), :])
            cw = pc.tile([128, 16], F32)
            nc.sync.dma_start(cw[:], cwt)
            cb = pc.tile([128, 4], F32)
            nc.sync.dma_start(cb[:], cbt)
            xp = pc.tile([128, 4, 160], BF16)
            nc.sync.dma_start(
                xp[:], xpt.rearrange("(k p) m -> p k m", p=128))
            onr = pc.tile([1, 128], F32R)
            nc.sync.dma_start(onr[:], ones_r.bitcast(F32R))
            onc = pc.tile([128, 1], F32R)
            nc.sync.dma_start(onc[:], ones_c.bitcast(F32R))

            # --- sum of squares over d (PE reduction with ones), rmsnorm scale
            ps_ss = pps.tile([1, L], F32)
            for k in range(KT):
                sq = psq.tile([128, L], F32R)
                nc.scalar.activation(sq[:], xsb[:, k, :], AF.Square)
                for h in range(2):
                    nc.tensor.matmul(
                        ps_ss[:, h * 512:(h + 1) * 512], onc[:],
                        sq[:, h * 512:(h + 1) * 512],
                        start=(k == 0), stop=(k == KT - 1))
            eps_c = pc.tile([1, 1], F32)
            nc.vector.memset(eps_c[:], EPS)
            sv = pc.tile([1, L], F32)
            nc.scalar.activation(sv[:], ps_ss[:], AF.Sqrt, bias=eps_c[:],
                                 scale=1.0 / DMODEL)
            s0 = pc.tile([1, L], F32)
            nc.vector.reciprocal(s0[:], sv[:])
            s0r = pc.tile([1, L], F32R)
            nc.scalar.copy(s0r[:], s0[:])
            s_rep = pc.tile([128, L], F32)
            for h in range(2):
                ps_sr = pp.tile([128, 512], F32, tag="mm")
                nc.tensor.matmul(ps_sr[:], onr[:],
                                 s0r[:, h * 512:(h + 1) * 512],
                                 start=True, stop=True)
                nc.scalar.copy(s_rep[:, h * 512:(h + 1) * 512], ps_sr[:])

            # --- in_proj (xc half) + causal conv + silu
            xc_tiles = []
            for m in range(4):
                xzp = pxz.tile([128, L + 4], BF16)
                nc.vector.memset(xzp[:, 0:4], 0.0)
                for h in range(2):
                    ps = pp.tile([128, 512], F32, tag="mm")
                    for k in range(KT):
                        nc.tensor.matmul(
                            ps[:], w1[:, k, m * 128:(m + 1) * 128],
                            xsb[:, k, h * 512:(h + 1) * 512],
                            start=(k == 0), stop=(k == KT - 1))
                    nc.vector.tensor_tensor(
                        xzp[:, 4 + h * 512: 4 + (h + 1) * 512], ps[:],
                        s_rep[:, h * 512:(h + 1) * 512], OP.mult)
                # conv taps: acc = sum_k w_k * xzp[:, k:k+L]
                c0 = pcv.tile([128, L], BF16, tag="cv")
                nc.vector.tensor_scalar_mul(c0[:], xzp[:, 1:1 + L],
                                            cw[:, 4 * m + 0: 4 * m + 1])
                c1 = pcv.tile([128, L], BF16, tag="cv")
                nc.vector.scalar_tensor_tensor(
                    c1[:], xzp[:, 2:2 + L], cw[:, 4 * m + 1: 4 * m + 2],
                    c0[:], OP.mult, OP.add)
                c2 = pcv.tile([128, L], BF16, tag="cv")
                nc.vector.scalar_tensor_tensor(
                    c2[:], xzp[:, 3:3 + L], cw[:, 4 * m + 2: 4 * m + 3],
                    c1[:], OP.mult, OP.add)
                c3 = pcv.tile([128, L], BF16, tag="cv")
                nc.vector.scalar_tensor_tensor(
                    c3[:], xzp[:, 4:4 + L], cw[:, 4 * m + 3: 4 * m + 4],
                    c2[:], OP.mult, OP.add)
                xc_m = pxc.tile([128, L], BF16)
                nc.scalar.activation(xc_m[:], c3[:], AF.Silu,
                                     bias=cb[:, m:m + 1])
                nc.scalar.dma_start(xc_out[m * 128:(m + 1) * 128, :],
                                     xc_m[:])
                xc_tiles.append(xc_m)

            # --- x_proj partial: xdp[r, t] = sum_d xpt[d, r] * xc[d, t]
            for h in range(2):
                pa = pp.tile([128, 512], F32, tag="mm")
                pb = ppb.tile([32, 512], F32)
                for kk in range(4):
                    nc.tensor.matmul(pa[:], xp[:, kk, 0:128],
                                     xc_tiles[kk][:, h * 512:(h + 1) * 512],
                                     start=(kk == 0), stop=(kk == 3))
                    nc.tensor.matmul(pb[:], xp[:, kk, 128:160],
                                     xc_tiles[kk][:, h * 512:(h + 1) * 512],
                                     start=(kk == 0), stop=(kk == 3))
                xda = pxd.tile([128, 512], F32, tag="xda")
                nc.scalar.copy(xda[:], pa[:])
                nc.scalar.dma_start(xdp_out[0:128, h * 512:(h + 1) * 512],
                                     xda[:])
                xdb = pxd.tile([32, 512], F32, tag="xdb")
                nc.scalar.copy(xdb[:], pb[:])
                nc.scalar.dma_start(xdp_out[128:160, h * 512:(h + 1) * 512],
                                     xdb[:])

            # --- in_proj (res half) + silu -> gate g
            w2 = pw.tile([128, KT, DL], BF16, tag="w")
            nc.scalar.dma_start(
                w2[:], w2t.rearrange("(k p) m -> p k m", p=128))
            for m in range(4):
                res_m = pres.tile([128, L], BF16, tag="res")
                for h in range(2):
                    ps = pp.tile([128, 512], F32, tag="mm")
                    for k in range(KT):
                        nc.tensor.matmul(
                            ps[:], w2[:, k, m * 128:(m + 1) * 128],
                            xsb[:, k, h * 512:(h + 1) * 512],
                            start=(k == 0), stop=(k == KT - 1))
                    nc.vector.tensor_tensor(
                        res_m[:, h * 512:(h + 1) * 512], ps[:],
                        s_rep[:, h * 512:(h + 1) * 512], OP.mult)
                g_m = pres.tile([128, L], BF16, tag="g")
                nc.scalar.activation(g_m[:], res_m[:], AF.Silu)
                nc.scalar.dma_start(g_out[m * 128:(m + 1) * 128, :], g_m[:])

    _split_sync_waits(nc)
    return nc


# ============================================================================
# Phase 2
# ============================================================================

def _build_phase2():
    nc = _new_nc()
    xc_in = nc.dram_tensor("xc_in", [DL, L], BF16, kind="ExternalInput").ap()
    g_in = nc.dram_tensor("g_in", [DL, L], BF16, kind="ExternalInput").ap()
    dl_in = nc.dram_tensor("dl_in", [DTR, L], F32, kind="ExternalInput").ap()
    brep = nc.dram_tensor("brep", [128, L], BF16, kind="ExternalInput").ap()
    crep = nc.dram_tensor("crep", [128, L], BF16, kind="ExternalInput").ap()
    dtt = nc.dram_tensor("dtt", [DTR, DL], F32, kind="ExternalInput").ap()
    dtbc = nc.dram_tensor("dtbc", [128, 4], F32, kind="ExternalInput").ap()
    acol = nc.dram_tensor("acol", [128, 64], F32, kind="ExternalInput").ap()
    dcol = nc.dram_tensor("dcol", [128, 4], F32, kind="ExternalInput").ap()
    rcol = nc.dram_tensor("rcol", [128, 2048], BF16, kind="ExternalInput").ap()
    scol = nc.dram_tensor("scol", [128, 2048], BF16, kind="ExternalInput").ap()
    wot = nc.dram_tensor("wot", [DL, DMODEL], BF16, kind="ExternalInput").ap()
    du_bounce = nc.dram_tensor("du_bounce", [DL, L], BF16).ap()
    yp_out = nc.dram_tensor("yp_out", [DMODEL, L], F32, kind="ExternalOutput").ap()

    with tile.TileContext(nc) as tc:
        with (
            tc.tile_pool(name="pc", bufs=1) as pc,
            tc.tile_pool(name="pu", bufs=1) as pu,
            tc.tile_pool(name="pdel", bufs=2) as pdel,
            tc.tile_pool(name="pscan", bufs=3) as pscan,
            tc.tile_pool(name="pyg", bufs=4) as pyg,
            tc.tile_pool(name="pwo", bufs=3) as pwo,
            tc.tile_pool(name="pyp", bufs=3) as pyp,
            tc.tile_pool(name="ps512", bufs=2, space="PSUM") as ps512,
            tc.tile_pool(name="ppr", bufs=2, space="PSUM") as ppr,
            tc.tile_pool(name="py", bufs=1, space="PSUM") as py,
        ):
            u4 = pu.tile([128, 4, L], BF16)
            nc.sync.dma_start(u4[:], xc_in.rearrange("(m p) t -> p m t", p=128))
            g4 = pu.tile([128, 4, L], BF16)
            nc.sync.dma_start(g4[:], g_in.rearrange("(m p) t -> p m t", p=128))
            dlsb = pc.tile([128, L], F32R)
            nc.sync.dma_start(dlsb[:], dl_in.bitcast(F32R))
            br = pc.tile([128, L], BF16)
            nc.sync.dma_start(br[:], brep)
            cr = pc.tile([128, L], BF16)
            nc.sync.dma_start(cr[:], crep)
            dt_sb = pc.tile([128, DL], F32R)
            nc.sync.dma_start(dt_sb[:], dtt.bitcast(F32R))
            dtb_sb = pc.tile([128, 4], F32)
            nc.sync.dma_start(dtb_sb[:], dtbc)
            a_sb = pc.tile([128, 64], F32)
            nc.sync.dma_start(a_sb[:], acol)
            d_sb = pc.tile([128, 4], F32)
            nc.sync.dma_start(d_sb[:], dcol)
            r_sb = pc.tile([128, 2048], BF16)
            nc.sync.dma_start(r_sb[:], rcol)
            s_sb = pc.tile([128, 2048], BF16)
            nc.sync.dma_start(s_sb[:], scol)

            # --- scan over 64 (d,n)-tiles; y accumulated per d-tile in PSUM
            yg_tiles = []
            for m in range(4):
                # dt_proj + softplus -> delta (bf16); du = delta * u (bf16)
                delta_m = pdel.tile([128, L], BF16, tag="delta")
                sp_e = pdel.tile([128, L], F32, tag="sp")
                for h in range(2):
                    ps = ps512.tile([128, 512], F32, tag="mm512")
                    nc.tensor.matmul(ps[:],
                                     dt_sb[:, m * 128:(m + 1) * 128],
                                     dlsb[:, h * 512:(h + 1) * 512],
                                     start=True, stop=True)
                    # softplus = ln(1 + exp(lin + bias)) via Exp then Ln
                    nc.scalar.activation(sp_e[:, h * 512:(h + 1) * 512],
                                         ps[:], AF.Exp,
                                         bias=dtb_sb[:, m:m + 1])
                nc.scalar.activation(delta_m[:], sp_e[:], AF.Ln, bias=1.0)
                du_m = pdel.tile([128, L], BF16, tag="du")
                nc.vector.tensor_tensor(du_m[:], delta_m[:], u4[:, m, :],
                                        OP.mult)
                nc.sync.dma_start(du_bounce[m * 128:(m + 1) * 128, :],
                                  du_m[:])

                ypsum = py.tile([128, L], F32)
                for v in range(16):
                    j = m * 16 + v
                    # replicate delta (PE, bf16) -> PSUM; exp on ACT
                    dA = pscan.tile([128, L], BF16, tag="dA")
                    pr = ppr.tile([128, L], F32, tag="rep_d")
                    for h in range(2):
                        nc.tensor.matmul(
                            pr[:, h * 512:(h + 1) * 512],
                            r_sb[:, v * 128:(v + 1) * 128],
                            delta_m[:, h * 512:(h + 1) * 512],
                            start=True, stop=True)
                    nc.scalar.activation(dA[:], pr[:], AF.Exp,
                                         scale=a_sb[:, j:j + 1])
                    # du replicated via broadcast DMA from DRAM
                    du_rep = pscan.tile([128, L], BF16, tag="du_rep")
                    nc.sync.dma_start(
                        du_rep[:],
                        du_bounce[m * 128 + 8 * v: m * 128 + 8 * v + 8, :]
                        .rearrange("d (one t) -> d one t", one=1)
                        .to_broadcast([8, 16, L]))
                    dBu = pscan.tile([128, L], BF16, tag="dBu")
                    nc.vector.tensor_tensor(dBu[:], du_rep[:], br[:], OP.mult)
                    # the scan itself: h[t] = dA[t]*h[t-1] + dBu[t]
                    hh = pscan.tile([128, L], BF16, tag="h")
                    nc.vector.tensor_tensor_scan(hh[:], dA[:], dBu[:], 0.0,
                                                 OP.mult, OP.add)
                    # h * C on gpsimd
                    hc = pscan.tile([128, L], BF16, tag="hc")
                    nc.gpsimd.tensor_tensor(hc[:], hh[:], cr[:], OP.mult)
                    # n-reduction into y (PE select-accumulate)
                    for h in range(2):
                        nc.tensor.matmul(
                            ypsum[:, h * 512:(h + 1) * 512],
                            s_sb[:, v * 128:(v + 1) * 128],
                            hc[:, h * 512:(h + 1) * 512],
                            start=(v == 0), stop=(v == 15))
                # y + u*D, then gate
                ya = pyg.tile([128, L], F32, tag="ya")
                for h in range(2):
                    nc.vector.scalar_tensor_tensor(
                        ya[:, h * 512:(h + 1) * 512],
                        u4[:, m, h * 512:(h + 1) * 512], d_sb[:, m:m + 1],
                        ypsum[:, h * 512:(h + 1) * 512], OP.mult, OP.add)
                yg = pyg.tile([128, L], BF16, tag="yg")
                nc.gpsimd.tensor_tensor(yg[:], ya[:], g4[:, m, :], OP.mult)
                yg_tiles.append(yg)

            # --- out_proj partial: yp[j, t] = sum_d wot[d, j] * yg[d, t]
            wot_r = wot.rearrange("(k p) m -> p k m", p=128)
            for mo in range(16):
                wo_mo = pwo.tile([128, 4, 128], BF16, tag="wo")
                nc.sync.dma_start(wo_mo[:],
                                  wot_r[:, :, mo * 128:(mo + 1) * 128])
                yp_m = pyp.tile([128, L], F32)
                for h in range(2):
                    po = ps512.tile([128, 512], F32, tag="mm512")
                    for k in range(4):
                        nc.tensor.matmul(
                            po[:], wo_mo[:, k, :],
                            yg_tiles[k][:, h * 512:(h + 1) * 512],
                            start=(k == 0), stop=(k == 3))
                    nc.scalar.copy(yp_m[:, h * 512:(h + 1) * 512], po[:])
                nc.scalar.dma_start(yp_out[mo * 128:(mo + 1) * 128, :], yp_m[:])

    _split_sync_waits(nc)
    return nc


# ============================================================================
# Host orchestration
# ============================================================================

_CACHE = {}


def _get_nc(which):
    if which not in _CACHE:
        _CACHE[which] = _build_phase1() if which == 1 else _build_phase2()
    return _CACHE[which]


def _c(a):
    return np.ascontiguousarray(a, dtype=np.float32)


def _cb(a):
    import ml_dtypes
    return np.ascontiguousarray(np.asarray(a, np.float32),
                                dtype=ml_dtypes.bfloat16)


def _sel_cols(vec512):
    # (512,) -> (128, 4): column m holds entries [m*128:(m+1)*128]
    return _c(vec512.reshape(4, 128).T)


def kernel(x, norm_w, in_proj_w, conv_w, conv_b, x_proj_w, dt_proj_w,
           dt_proj_b, A_log, D, out_proj_w, trace=False):
    D_ = D
    x = np.asarray(x, dtype=np.float32)
    b, l, d = x.shape
    assert (b, l, d) == (1, L, DMODEL)
    x2d = x[0]
    xTb = _cb(x2d.T)

    norm_w = np.asarray(norm_w, np.float32)
    in_proj_w = np.asarray(in_proj_w, np.float32)
    W_norm = in_proj_w * norm_w[None, :]

    A = -np.exp(np.asarray(A_log, np.float32))       # (DIN, NST)
    conv_w2 = np.asarray(conv_w, np.float32)[:, 0, :]  # (DIN, 4)
    conv_b = np.asarray(conv_b, np.float32)
    x_proj_w = np.asarray(x_proj_w, np.float32)
    dt_proj_w = np.asarray(dt_proj_w, np.float32)
    dt_proj_b = np.asarray(dt_proj_b, np.float32)
    D_vec = np.asarray(D_, np.float32)
    out_proj_w = np.asarray(out_proj_w, np.float32)

    # ---- phase 1 inputs
    in_maps1 = []
    for c in range(NCORES):
        sl = slice(c * DL, (c + 1) * DL)
        cw = conv_w2[sl]  # (512, 4)
        cwt = _c(cw.reshape(4, 128, 4).transpose(1, 0, 2).reshape(128, 16))
        in_maps1.append(dict(
            xt=xTb,
            w1t=_cb(W_norm[sl, :].T),
            w2t=_cb(W_norm[DIN + c * DL: DIN + (c + 1) * DL, :].T),
            xpt=_cb(x_proj_w[:, sl].T),
            cwt=cwt,
            cbt=_sel_cols(conv_b[sl]),
            ones_r=np.ones((1, 128), np.float32),
            ones_c=np.ones((128, 1), np.float32),
        ))
    res1 = run_bass_kernel_spmd(_get_nc(1), in_maps1, list(range(NCORES)),
                                trace=trace,
                                trace_cores=list(range(NCORES)) if trace else None)

    # ---- host "all-reduce" of partial x_dbl
    xdb = np.zeros((160, L), np.float32)
    for c in range(NCORES):
        xdb += res1.results[c]["xdp_out"]
    dl_full = _c(xdb[:DTR])            # (128, L)
    B = xdb[DTR:DTR + NST]             # (16, L)
    C = xdb[DTR + NST:DTR + 2 * NST]   # (16, L)
    brep_np = _cb(np.tile(B, (8, 1)))
    crep_np = _cb(np.tile(C, (8, 1)))

    # replication / selection 0-1 matrices (shared by all cores)
    rcol = np.zeros((128, 16, 128), np.float32)
    scol = np.zeros((128, 16, 128), np.float32)
    for v in range(16):
        for mm in range(128):
            rcol[8 * v + mm // 16, v, mm] = 1.0
        for p in range(128):
            scol[p, v, 8 * v + p // 16] = 1.0
    rcol = _cb(rcol.reshape(128, 2048))
    scol = _cb(scol.reshape(128, 2048))

    # ---- phase 2 inputs
    in_maps2 = []
    for c in range(NCORES):
        sl = slice(c * DL, (c + 1) * DL)
        in_maps2.append(dict(
            xc_in=res1.results[c]["xc_out"],
            g_in=res1.results[c]["g_out"],
            dl_in=dl_full,
            brep=brep_np,
            crep=crep_np,
            dtt=_c(dt_proj_w[sl, :].T),
            dtbc=_sel_cols(dt_proj_b[sl]),
            acol=_c(A[sl].reshape(DL * NST).reshape(64, 128).T),
            dcol=_sel_cols(D_vec[sl]),
            rcol=rcol,
            scol=scol,
            wot=_cb(out_proj_w[:, sl].T),
        ))
    res2 = run_bass_kernel_spmd(_get_nc(2), in_maps2, list(range(NCORES)),
                                trace=trace,
                                trace_cores=list(range(NCORES)) if trace else None)

    # ---- host reduce of partial out_proj + residual
    acc = np.zeros((DMODEL, L), np.float32)
    for c in range(NCORES):
        acc += res2.results[c]["yp_out"]
    out = acc.T + x2d
    if trace:
        kernel.last_exec_times = (res1.exec_time_ns, res2.exec_time_ns)
        kernel.last_results = (res1, res2)
    return out.reshape(1, L, DMODEL).astype(np.float32)

